# revision 22
# baseline (speedup 1.0000x reference)
"""GAT-style 3-layer GNN on 8 Trainium2 NeuronCores.

Math: per layer h = leaky_relu(h@W+b); softmax over e_ij = sL[i]+sR[j]+c is
invariant to the per-source terms, so alpha_ij = exp(sR[j]) / sum_{j' in N(i)}
exp(sR[j']).  The edge phase therefore reduces to a segment-sum over edges of
the per-node payload A[j] = [w_j * h_j | w_j] with w = exp(sR), followed by
h2[i] = num/den and relu.

Sharding: core k owns source nodes [6250k, 6250(k+1)) and all their edges.
Each core's segments are bin-packed into 49 windows x 128 partition slots
(a per-core node permutation baked into all index data on the host).  The
edge payload table A (fp16, 512B rows) is rebuilt per layer from the dense
phase and AllGather-ed; edges gather rows of A with dma_gather (int16 idx,
split lo/hi at row 32768), one-hot scatter masks are built on DVE with an
iota/is_equal trick, and PE matmuls S.T @ G accumulate [num|den] per window
in PSUM.
"""

import math
import numpy as np

import concourse.bass as bass
import concourse.bacc as bacc
import concourse.mybir as mybir
import concourse.tile as tile
from concourse.bass_utils import run_bass_kernel_spmd
from concourse.library_config import mlp as _mlp_lib

F32 = mybir.dt.float32
F16 = mybir.dt.float16
I16 = mybir.dt.int16
AF = mybir.ActivationFunctionType

N_NODES = 50000
N_EDGES = 800000
D = 128
N_LAYERS = 3
N_CORES = 8
SEGS_PER_CORE = N_NODES // N_CORES  # 6250


def _pack_windows(seg_lo, seg_hi, n_win):
    """Assign each segment to a window (<=128 segs per window), balancing lo
    and hi edge loads.  Returns win[seg], per-window seg lists."""
    nseg = len(seg_lo)
    order = np.argsort(-(seg_lo + seg_hi), kind="stable")
    lo_load = np.zeros(n_win)
    hi_load = np.zeros(n_win)
    cnt = np.zeros(n_win, dtype=np.int64)
    tgt_lo = max(seg_lo.sum() / n_win, 1.0)
    tgt_hi = max(seg_hi.sum() / n_win, 1.0)
    win_of = np.zeros(nseg, dtype=np.int64)
    for s in order:
        score = np.maximum((lo_load + seg_lo[s]) / tgt_lo, (hi_load + seg_hi[s]) / tgt_hi)
        score[cnt >= 128] = np.inf
        wbest = int(np.argmin(score))
        win_of[s] = wbest
        lo_load[wbest] += seg_lo[s]
        hi_load[wbest] += seg_hi[s]
        cnt[wbest] += 1
    return win_of


def _build_program(n_win, cl, ch, npad_core, npad_all, lo_split):
    """Build the SPMD bass program (identical on all cores)."""
    nc = bacc.Bacc(None, num_devices=N_CORES, debug=False)
    CPW = cl + ch  # chunks per window
    NCOLS = n_win * CPW * 128 // 16  # idx sbuf columns
    n_dense = (npad_core + 511) // 512  # 512-col dense blocks
    hi_rows = npad_all - lo_split

    xT_d = nc.dram_tensor("xT", [128, npad_core], F32, kind="ExternalInput")
    gidx_d = nc.dram_tensor("gidx", [128, NCOLS], I16, kind="ExternalInput")
    segrel_d = nc.dram_tensor("segrel", [128, n_win * CPW], F32, kind="ExternalInput")
    iota_d = nc.dram_tensor("iota", [128, 128], F16, kind="ExternalInput")
    ident_d = nc.dram_tensor("ident", [128, 128], F32, kind="ExternalInput")
    wmat_d = nc.dram_tensor("wmat", [128, 3 * 128], F32, kind="ExternalInput")
    bvec_d = nc.dram_tensor("bvec", [128, 3], F32, kind="ExternalInput")
    wr_d = nc.dram_tensor("wr", [128, 3], F32, kind="ExternalInput")
    out_d = nc.dram_tensor("out", [npad_core, 128], F32, kind="ExternalOutput")

    with tile.TileContext(nc) as tc:
        with (
            tc.tile_pool(name="const", bufs=1) as constp,
            tc.tile_pool(name="big", bufs=1) as bigp,
            tc.tile_pool(name="hown", bufs=2) as hop,
            tc.tile_pool(name="glo", bufs=3) as glop,
            tc.tile_pool(name="ghi", bufs=3) as ghip,
            tc.tile_pool(name="mask", bufs=4) as maskp,
            tc.tile_pool(name="small", bufs=3) as smallp,
            tc.tile_pool(name="one", bufs=1) as onep,
            tc.tile_pool(name="asb", bufs=3) as asbp,
            tc.tile_pool(name="mm", bufs=2, space="PSUM") as mmp,
            tc.tile_pool(name="tp", bufs=2, space="PSUM") as tpp,
            tc.tile_pool(name="ep", bufs=3, space="PSUM") as epp,
            tc.tile_pool(name="dram", bufs=1, space="DRAM") as dramp,
        ):
            nc.gpsimd.load_library(_mlp_lib)
            # split each window's gather into calls of <= GMAX chunks (large
            # single dma_gather calls overflow the SWDGE descriptor ring and
            # deadlock on HW); one shared num_idxs register per distinct size
            GMAX = 5
            regs = {}
            for n in {min(GMAX, cl), cl % GMAX or GMAX, min(GMAX, ch), ch % GMAX or GMAX}:
                if n > 0:
                    regs[n] = nc.gpsimd.to_reg(n * 128)

            def gather_split(tile_t, src_ap, col0, nck):
                """Gather nck chunks (128 idxs each) starting at idx column
                col0 into tile_t, in GMAX-chunk slices."""
                for s0 in range(0, nck, GMAX):
                    s1 = min(s0 + GMAX, nck)
                    nc.gpsimd.dma_gather(
                        tile_t[:, s0:s1, :], src_ap,
                        gidx[:, col0 + s0 * 8 : col0 + s1 * 8],
                        (s1 - s0) * 128, regs[s1 - s0], 256, elem_step=256,
                    )
            # constants
            gidx = constp.tile([128, NCOLS], I16)
            segrel = constp.tile([128, n_win * CPW], F32)
            iota = constp.tile([128, 128], F16)
            ident = constp.tile([128, 128], F32)
            wmat = constp.tile([128, 3 * 128], F32)
            bvec = constp.tile([128, 3], F32)
            wr = constp.tile([128, 3], F32)
            nc.sync.dma_start(gidx[:], gidx_d[:])
            nc.sync.dma_start(segrel[:], segrel_d[:])
            nc.sync.dma_start(iota[:], iota_d[:])
            nc.sync.dma_start(ident[:], ident_d[:])
            nc.sync.dma_start(wmat[:], wmat_d[:])
            nc.sync.dma_start(bvec[:], bvec_d[:])
            nc.sync.dma_start(wr[:], wr_d[:])

            a_own = dramp.tile([npad_core, 256], F16)
            a_full = dramp.tile([npad_all, 256], F16)
            wrow_dram = dramp.tile([1, npad_core], F32)

            # zero the pad columns of a_own once (cols 129:256 are never
            # written by the A-build but are carried by the AllGather)
            zp = onep.tile([128, 127], F16, tag="zpad")
            nc.vector.memset(zp[:], 0.0)
            for w in range(n_win):
                nc.sync.dma_start(a_own[w * 128 : (w + 1) * 128, 129:256], zp[:])

            h_own = None  # layout: [128 slots(part), n_win, 128 feat]

            for l in range(N_LAYERS):
                # ---- dense phase: hT [feat, node] ----
                hT = bigp.tile([128, npad_core], F32, tag="hT")
                if l == 0:
                    nc.sync.dma_start(hT[:], xT_d[:])
                else:
                    for w in range(n_win):
                        pt = tpp.tile([128, 128], F32, tag="tp")
                        nc.tensor.transpose(pt[:], h_own[:, w, :], ident[:])
                        nc.scalar.activation(hT[:, w * 128 : (w + 1) * 128], pt[:], AF.Copy)
                hnT = bigp.tile([128, npad_core], F32, tag="hnT")
                wrow = onep.tile([1, npad_core], F32, tag="wrow")
                for b in range(n_dense):
                    lo = b * 512
                    hi = min(lo + 512, npad_core)
                    pd = mmp.tile([128, 512], F32, tag="mm")
                    nc.tensor.matmul(
                        pd[:, : hi - lo], wmat[:, l * 128 : (l + 1) * 128], hT[:, lo:hi],
                        start=True, stop=True,
                    )
                    # leaky_relu(v + b) = max(v + b, 0.2*(v + b))
                    t1 = smallp.tile([128, 512], F32, tag="lr1")
                    nc.scalar.activation(
                        t1[:, : hi - lo], pd[:, : hi - lo], AF.Identity,
                        bias=bvec[:, l : l + 1],
                    )
                    nc.vector.tensor_scalar(
                        hnT[:, lo:hi], t1[:, : hi - lo], 0.2, None,
                        mybir.AluOpType.mult,
                    )
                    nc.vector.tensor_tensor(
                        hnT[:, lo:hi], t1[:, : hi - lo], hnT[:, lo:hi],
                        mybir.AluOpType.max,
                    )
                    ps = mmp.tile([1, 512], F32, tag="mm")
                    nc.tensor.matmul(
                        ps[:, : hi - lo], wr[:, l : l + 1], hnT[:, lo:hi],
                        start=True, stop=True,
                    )
                    nc.scalar.activation(wrow[:, lo:hi], ps[:, : hi - lo], AF.Exp)
                # transpose w to per-partition column layout via DRAM bounce
                nc.sync.dma_start(wrow_dram[:], wrow[:])
                wcol = smallp.tile([128, n_win], F32, tag="wcol")
                nc.sync.dma_start(
                    wcol[:], wrow_dram[:].rearrange("o (w p) -> (o p) w", p=128)
                )
                # build A rows: [w*h | w] fp16, node-major
                for w in range(n_win):
                    pa = tpp.tile([128, 128], F32, tag="tp")
                    nc.tensor.transpose(pa[:], hnT[:, w * 128 : (w + 1) * 128], ident[:])
                    asb = asbp.tile([128, 129], F16, tag="asb")
                    nc.scalar.activation(
                        asb[:, 0:128], pa[:], AF.Copy, scale=wcol[:, w : w + 1]
                    )
                    nc.scalar.activation(
                        asb[:, 128:129], wcol[:, w : w + 1], AF.Copy
                    )
                    nc.sync.dma_start(a_own[w * 128 : (w + 1) * 128, 0:129], asb[:])
                # ---- exchange ----
                nc.gpsimd.collective_compute(
                    "AllGather",
                    mybir.AluOpType.bypass,
                    replica_groups=[list(range(N_CORES))],
                    ins=[a_own[:].opt()],
                    outs=[a_full[:].opt()],
                )
                # ---- edge phase ----
                h_new = hop.tile([128, n_win, 128], F32, tag="hown")
                a_lo = a_full[0:lo_split, :]
                a_hi = a_full[lo_split:npad_all, :]
                for w in range(n_win):
                    base = w * CPW * 128 // 16
                    glo_t = glop.tile([128, cl, 256], F16, tag="glo")
                    gather_split(glo_t, a_lo, base, cl)
                    if ch > 0:
                        ghi_t = ghip.tile([128, ch, 256], F16, tag="ghi")
                        gather_split(ghi_t, a_hi, base + cl * 8, ch)
                    pe = epp.tile([128, 129], F32, tag="ep")
                    for c in range(CPW):
                        sm = maskp.tile([128, 128], F16, tag="mask")
                        nc.vector.tensor_scalar(
                            sm[:], iota[:], segrel[:, w * CPW + c : w * CPW + c + 1],
                            None, mybir.AluOpType.is_equal,
                        )
                        g = glo_t[:, c, 0:129] if c < cl else ghi_t[:, c - cl, 0:129]
                        nc.tensor.matmul(
                            pe[:], sm[:], g, start=(c == 0), stop=(c == CPW - 1)
                        )
                    rd = smallp.tile([128, 1], F32, tag="rd")
                    nc.vector.reciprocal(rd[:], pe[:, 128:129])
                    nc.scalar.activation(
                        h_new[:, w, :], pe[:, 0:128], AF.Relu, scale=rd[:]
                    )
                h_own = h_new
            # ---- output ----
            nc.sync.dma_start(
                out_d[:].rearrange("(w p) f -> p w f", p=128), h_own[:]
            )
    nc.compile()
    return nc


def _preprocess(x, edge_src, edge_dst, lin_w, lin_b, attn_w,
                n_nodes, n_cores, n_win, lo_split):
    segs_per_core = n_nodes // n_cores
    npad_core = n_win * 128
    npad_all = npad_core * n_cores

    edge_src = np.asarray(edge_src).astype(np.int64)
    edge_dst = np.asarray(edge_dst).astype(np.int64)
    x = np.asarray(x, dtype=np.float32)

    core_of = edge_src // segs_per_core
    # first pass: per-core window packing needs global dst positions, which
    # need every core's slot assignment -> two passes.
    win_all, slot_all = [], []
    for k in range(n_cores):
        m = core_of == k
        src_loc = edge_src[m] - k * segs_per_core
        # lo/hi split for packing balance only; use original-id proxy
        nseg = segs_per_core
        lo_mask_proxy = edge_dst[m] < (lo_split * n_nodes // npad_all)
        seg_lo = np.bincount(src_loc[lo_mask_proxy], minlength=nseg)
        seg_hi = np.bincount(src_loc[~lo_mask_proxy], minlength=nseg)
        win_of = _pack_windows(seg_lo, seg_hi, n_win)
        slot_of = np.zeros(nseg, dtype=np.int64)
        nxt = np.zeros(n_win, dtype=np.int64)
        for s in range(nseg):
            w = win_of[s]
            slot_of[s] = nxt[w]
            nxt[w] += 1
        assert nxt.max() <= 128
        win_all.append(win_of)
        slot_all.append(slot_of)

    # global position of node n in the AllGather-ed A table
    pos = np.zeros(n_nodes, dtype=np.int64)
    for k in range(n_cores):
        ids = np.arange(segs_per_core)
        pos[k * segs_per_core + ids] = k * npad_core + win_all[k] * 128 + slot_all[k]

    dst_pos = pos[edge_dst]
    lo_mask = dst_pos < lo_split

    # per (core, window): lo/hi edge lists -> chunk counts
    CL = CH = 0
    per_core = []
    for k in range(n_cores):
        m = core_of == k
        src_loc = edge_src[m] - k * segs_per_core
        wof = win_all[k][src_loc]
        sof = slot_all[k][src_loc]
        dp = dst_pos[m]
        lom = lo_mask[m]
        wins = []
        for w in range(n_win):
            wm = wof == w
            lo_e = np.stack([dp[wm & lom], sof[wm & lom]], 1)
            hi_e = np.stack([dp[wm & ~lom] - lo_split, sof[wm & ~lom]], 1)
            wins.append((lo_e, hi_e))
            CL = max(CL, (len(lo_e) + 127) // 128)
            CH = max(CH, (len(hi_e) + 127) // 128)
        per_core.append(wins)

    CPW = CL + CH
    # build per-core gather idx + segrel arrays
    in_maps = []
    for k in range(n_cores):
        gidx = np.zeros((n_win * CPW * 128,), dtype=np.int16)
        segrel = np.full((128, n_win * CPW), 200.0, dtype=np.float32)
        for w in range(n_win):
            lo_e, hi_e = per_core[k][w]
            for side, (elist, c0, ccnt) in enumerate(
                [(lo_e, 0, CL), (hi_e, CL, CH)]
            ):
                n = len(elist)
                assert n <= ccnt * 128
                base = w * CPW * 128 + c0 * 128
                if n:
                    gidx[base : base + n] = elist[:, 0].astype(np.int16)
                    cc = c0 + np.arange(n) // 128
                    pp = np.arange(n) % 128
                    segrel[pp, w * CPW + cc] = elist[:, 1].astype(np.float32)
        # wrap indices: position i -> [i%16, i//16], replicate to 128 partitions
        gw = gidx.reshape(-1, 16).T  # [16, tot/16]
        gw = np.tile(gw, (8, 1))  # [128, tot/16]

        # xT for own nodes (permuted): slot (w,p) <- node with pos k*npad+w*128+p
        xT = np.zeros((128, npad_core), dtype=np.float32)
        ids = np.arange(segs_per_core)
        mypos = win_all[k] * 128 + slot_all[k]  # local position of local seg
        xT_cols = np.zeros((npad_core, x.shape[1]), dtype=np.float32)
        xT_cols[mypos] = x[k * segs_per_core + ids]
        xT = xT_cols.T.copy()

        in_maps.append(
            {
                "xT": xT,
                "gidx": np.ascontiguousarray(gw),
                "segrel": segrel,
                "iota": np.tile(np.arange(128, dtype=np.float16)[None, :], (128, 1)),
                "ident": np.eye(128, dtype=np.float32),
                "wmat": np.concatenate([lin_w[l] for l in range(3)], axis=1).astype(np.float32),
                "bvec": np.asarray(lin_b, dtype=np.float32).T.copy(),
                "wr": np.asarray(attn_w, dtype=np.float32)[:, x.shape[1]:].T.copy(),
            }
        )
    return in_maps, pos, CL, CH


def kernel(x, edge_src, edge_dst, lin_w, lin_b, attn_w, attn_b, _trace=False,
           _tmpdir=None):
    n_win = (SEGS_PER_CORE + 127) // 128  # 49
    npad_core = n_win * 128
    npad_all = npad_core * N_CORES
    lo_split = 32768

    in_maps, pos, CL, CH = _preprocess(
        x, edge_src, edge_dst, lin_w, lin_b, attn_w,
        N_NODES, N_CORES, n_win, lo_split,
    )
    nc = _build_program(n_win, CL, CH, npad_core, npad_all, lo_split)
    res = run_bass_kernel_spmd(
        nc, in_maps, list(range(N_CORES)), trace=_trace, tmpdir=_tmpdir,
    )
    outs = res.results if hasattr(res, "results") else res
    full = np.zeros((N_NODES, D), dtype=np.float32)
    for k in range(N_CORES):
        ok = outs[k]["out"]  # [npad_core, 128]
        ids = np.arange(SEGS_PER_CORE)
        mypos = pos[k * SEGS_PER_CORE + ids] - k * npad_core
        full[k * SEGS_PER_CORE + ids] = ok[mypos]
    if _trace:
        return full, res
    return full
